# revision 22
# baseline (speedup 1.0000x reference)
"""Gaussian kernel matrix K = exp(-|xi-xj|^2/2) on 8 TRN2 NeuronCores,
exploiting symmetry: each core computes only lower-triangle block rows.

Input : points [4, 4096, 64] f32
Output: K      [4, 4096, 4096] f32

K[b] is symmetric: only the lower block triangle (block row r of 128
rows spans columns 0..(r+1)*128) is computed on-device; the host
mirrors the strict upper triangle.

Sharding (SPMD-uniform): core c = (batch c//2, h = c%2) takes pairs
(r, 31-r) for r = h, h+2, ..., h+14.  Widths are padded up to 512
multiples, which makes the padded pair shape IDENTICAL for all cores:
pair slot q has lo width L_q=(q//2+1)*512, hi width H_q=4608-L_q, so
every pair strip is [128, 4608] and all 8 cores run the same program
(~4.61M padded outputs/core vs 8.39M for the full-matrix split).

Math: -d2/2 = xi.xj - |xi|^2/2 - |xj|^2/2, one fp16 matmul pass with
two augmented contraction rows (ones | fp16 hi/lo of -|xj|^2/2), K=66.
-|xi|^2/2 enters as the fp32 per-partition ScalarE bias:
K = Exp(psum + bias_i), emitted bf16 (host upcasts).
"""

import numpy as np

B, N, D = 4, 4096, 64
KA = D + 2          # contraction dim: 64 dims + hi/lo aug rows
N_CORES = 8
NBLK = N // 128     # 32 block rows per batch
NPAIR = 8           # pairs per core
PW = 4608           # padded columns per pair strip (L_q + H_q)
GPP = 3             # chunk groups per pair (lo: 1, hi: 2048 + rest)
NGRP = NPAIR * GPP  # 24 pipeline groups per core

NBUF_OUT = 6        # staging strips of [128, PW]

_cache = {}


def _lo_w(q):
    return (q // 2 + 1) * 512   # padded lo width: 512,512,1024,...,2048


def _pairs(h):
    return [(h + 2 * q, 31 - (h + 2 * q)) for q in range(NPAIR)]


def _groups():
    """Static schedule, identical for every core: one entry per
    (pair, block, col-chunk): bi = lhs block slot, c0 = rhs col start,
    cw = chunk width, off = staging col offset within the pair strip."""
    gs = []
    for q in range(NPAIR):
        L = _lo_w(q)
        H = PW - L
        gs.append(dict(q=q, bi=2 * q, c0=0, cw=L, off=0))
        gs.append(dict(q=q, bi=2 * q + 1, c0=0, cw=2048, off=L))
        gs.append(dict(q=q, bi=2 * q + 1, c0=2048, cw=H - 2048,
                       off=L + 2048))
    assert len(gs) == NGRP
    return gs


def _build_nc():
    import concourse.bass as bass
    import concourse.mybir as mybir

    f32 = mybir.dt.float32
    f16 = mybir.dt.float16
    bf16 = mybir.dt.bfloat16
    Exp = mybir.ActivationFunctionType.Exp

    groups = _groups()

    nc = bass.Bass()
    # packed operand tensor: cols 0:2048 = xl (16 lhs blocks, ones aug),
    # cols 2048:6144 = xr (all points + -sq/2 hi/lo aug)
    XL0, XR0 = 0, 2048
    xin_d = nc.dram_tensor("xin", [KA, XR0 + N], f16, kind="ExternalInput")
    bias_d = nc.dram_tensor("bias", [128, 16], f32, kind="ExternalInput")
    out_d = nc.dram_tensor("out", [NPAIR * 128, PW], bf16,
                           kind="ExternalOutput")

    with (
        nc.sbuf_tensor([KA, XR0 + N], f16) as xin,
        nc.sbuf_tensor([128, 16], f32) as bias,
        nc.sbuf_tensor([128, NBUF_OUT * PW], bf16) as ot_buf,
        nc.psum_tensor([128, 2048], f32) as ps0,
        nc.psum_tensor([128, 2048], f32) as ps1,
        nc.semaphore("in_sem") as in_sem,
        nc.semaphore("in2_sem") as in2_sem,
        nc.semaphore("in3_sem") as in3_sem,
        nc.semaphore("mm_sem") as mm_sem,
        nc.semaphore("act_sem") as act_sem,
        nc.semaphore("out_sem_a") as out_sem_a,
        nc.semaphore("out_sem_b") as out_sem_b,
        nc.Block() as block,
    ):
        pss = [ps0, ps1]

        @block.sync
        def _(sync):
            # input loads, fewest serial dma_starts on the critical path:
            # xl + xr-left (one packed transfer), then bias, then xr-right;
            # separate semaphores because completion order across batches
            # is not issue order
            sync.dma_start(out=xin[:, 0 : XR0 + 2048],
                           in_=xin_d[:, 0 : XR0 + 2048]).then_inc(in_sem, 16)
            sync.dma_start(out=bias[:], in_=bias_d[:, :]).then_inc(in3_sem, 16)
            sync.dma_start(out=xin[:, XR0 + 2048 : XR0 + N],
                           in_=xin_d[:, XR0 + 2048 : XR0 + N],
                           ).then_inc(in2_sem, 16)
            for p in range(NPAIR):
                # both output rings on the otherwise-idle SP engine: cols
                # 0:L+2048 after acts g0+g1, the g2 chunk after act g2
                sync.wait_ge(act_sem, GPP * p + 2)
                L = _lo_w(p)
                s = (p % NBUF_OUT) * PW
                sync.dma_start(
                    out=out_d[p * 128 : (p + 1) * 128, 0 : L + 2048],
                    in_=ot_buf[:, s : s + L + 2048],
                ).then_inc(out_sem_a, 16)
                sync.wait_ge(act_sem, GPP * (p + 1))
                sync.dma_start(
                    out=out_d[p * 128 : (p + 1) * 128, L + 2048 : PW],
                    in_=ot_buf[:, s + L + 2048 : s + PW],
                ).then_inc(out_sem_b, 16)

        @block.tensor
        def _(tensor):
            tensor.wait_ge(in_sem, 16)
            for g, gr in enumerate(groups):
                if g == 2:
                    tensor.wait_ge(in2_sem, 16)  # xr right half loaded
                if g >= 2:
                    # psum slot g%2 was last read by group g-2's activation
                    tensor.wait_ge(act_sem, g - 1)
                ps = pss[g % 2]
                lh = xin[:, gr["bi"] * 128 : (gr["bi"] + 1) * 128]
                last = None
                for c in range(0, gr["cw"], 512):
                    last = tensor.matmul(
                        ps[:, c : c + 512],
                        lh,
                        xin[:, XR0 + gr["c0"] + c : XR0 + gr["c0"] + c + 512],
                        start=True, stop=True,
                    )
                last.then_inc(mm_sem, 1)

        @block.scalar
        def _(scalar):
            scalar.wait_ge(in3_sem, 16)  # bias loaded
            for g, gr in enumerate(groups):
                ps = pss[g % 2]
                scalar.wait_ge(mm_sem, g + 1)
                p = gr["q"]
                if g % GPP == 0 and p >= NBUF_OUT:
                    # staging slot p%NBUF_OUT last read by DMAs of p-NBUF_OUT
                    scalar.wait_ge(out_sem_a, 16 * (p - NBUF_OUT + 1))
                    scalar.wait_ge(out_sem_b, 16 * (p - NBUF_OUT + 1))
                s = (p % NBUF_OUT) * PW + gr["off"]
                scalar.activation(
                    ot_buf[:, s : s + gr["cw"]], ps[:, 0 : gr["cw"]], Exp,
                    bias=bias[:, gr["bi"] : gr["bi"] + 1], scale=1.0,
                ).then_inc(act_sem, 1)
    return nc


def _get_nc():
    if "nc" not in _cache:
        _cache["nc"] = _build_nc()
    return _cache["nc"]


def _prep_inputs(points: np.ndarray):
    """Host-side shard/layout prep: per-core transposed + augmented operands."""
    points = np.asarray(points, dtype=np.float32)
    per_batch = {}
    for b in range(B):
        x = points[b]                              # [N, D]
        sq = np.sum(x * x, axis=1)                 # [N]
        xt = np.ascontiguousarray(x.T)             # [D, N]
        aug_hi = (-0.5 * sq).astype(np.float16)
        aug_lo = ((-0.5 * sq) - aug_hi.astype(np.float32)).astype(np.float16)
        xr = np.empty((KA, N), np.float16)
        xr[:D] = xt
        xr[D] = aug_hi
        xr[D + 1] = aug_lo
        per_batch[b] = (xr, sq)

    in_maps = []
    for c in range(N_CORES):
        b, h = divmod(c, 2)
        xr, sq = per_batch[b]
        xin = np.empty((KA, 2048 + N), np.float16)
        xin[:, 2048:] = xr
        bias = np.empty((128, 16), np.float32)
        for q, (r_lo, r_hi) in enumerate(_pairs(h)):
            for slot, r in ((2 * q, r_lo), (2 * q + 1, r_hi)):
                rows = slice(r * 128, (r + 1) * 128)
                xin[:D, slot * 128 : (slot + 1) * 128] = xr[:D, rows]
                bias[:, slot] = -0.5 * sq[rows]
        xin[D, 0:2048] = 1.0
        xin[D + 1, 0:2048] = 1.0
        in_maps.append({"xin": xin, "bias": bias})
    return in_maps


def _assemble(results):
    """Unpack per-core strips, mirror the strict upper block triangle."""
    out = np.empty((B, N, N), np.float32)
    for c in range(N_CORES):
        b, h = divmod(c, 2)
        buf = results[c]["out"].astype(np.float32)   # [1024, PW]
        for q, (r_lo, r_hi) in enumerate(_pairs(h)):
            L = _lo_w(q)
            w_lo = (r_lo + 1) * 128
            w_hi = (r_hi + 1) * 128
            rows = buf[q * 128 : (q + 1) * 128]
            out[b, r_lo * 128 : (r_lo + 1) * 128, 0:w_lo] = rows[:, 0:w_lo]
            out[b, r_hi * 128 : (r_hi + 1) * 128, 0:w_hi] = \
                rows[:, L : L + w_hi]
    iu, ju = np.triu_indices(NBLK, 1)
    for b in range(B):
        v = out[b].reshape(NBLK, 128, NBLK, 128)
        v[iu, :, ju, :] = v[ju, :, iu, :].transpose(0, 2, 1)
    return out


def run(points: np.ndarray, **run_kwargs):
    """Run on HW; returns (K [4,4096,4096] f32, BassKernelResults)."""
    from concourse.bass_utils import run_bass_kernel_spmd

    nc = _get_nc()
    in_maps = _prep_inputs(points)
    res = run_bass_kernel_spmd(nc, in_maps, core_ids=list(range(N_CORES)),
                               **run_kwargs)
    return _assemble(res.results), res


def kernel(points: np.ndarray) -> np.ndarray:
    out, _ = run(points)
    return out


# revision 23
# speedup vs baseline: 1.1896x; 1.1896x over previous
"""Gaussian kernel matrix K = exp(-|xi-xj|^2/2) on 8 TRN2 NeuronCores,
exploiting symmetry: each core computes only lower-triangle block rows.

Input : points [4, 4096, 64] f32
Output: K      [4, 4096, 4096] f32

K[b] is symmetric: only the lower block triangle (block row r of 128
rows spans columns 0..(r+1)*128) is computed on-device; the host
mirrors the strict upper triangle.

Sharding (SPMD-uniform): core c = (batch c//2, h = c%2) takes pairs
(r, 31-r) for r = h, h+2, ..., h+14.  Widths are padded up to 512
multiples, which makes the padded pair shape IDENTICAL for all cores:
pair slot q has lo width L_q=(q//2+1)*512, hi width H_q=4608-L_q, so
every pair strip is [128, 4608] and all 8 cores run the same program
(~4.61M padded outputs/core vs 8.39M for the full-matrix split).

Math: -d2/2 = xi.xj - |xi|^2/2 - |xj|^2/2, one fp16 matmul pass with
two augmented contraction rows (ones | fp16 hi/lo of -|xj|^2/2), K=66.
-|xi|^2/2 enters as the fp32 per-partition ScalarE bias:
K = Exp(psum + bias_i), emitted bf16 (host upcasts).
"""

import numpy as np

B, N, D = 4, 4096, 64
KA = D + 2          # contraction dim: 64 dims + hi/lo aug rows
N_CORES = 8
NBLK = N // 128     # 32 block rows per batch
NPAIR = 8           # pairs per core
PW = 4608           # padded columns per pair strip (L_q + H_q)
GPP = 3             # chunk groups per pair (lo: 1, hi: 2048 + rest)
NGRP = NPAIR * GPP  # 24 pipeline groups per core

NBUF_OUT = 6        # staging strips of [128, PW]

_cache = {}


def _lo_w(q):
    return (q // 2 + 1) * 512   # padded lo width: 512,512,1024,...,2048


def _pairs(h):
    return [(h + 2 * q, 31 - (h + 2 * q)) for q in range(NPAIR)]


def _groups():
    """Static schedule, identical for every core: one entry per
    (pair, block, col-chunk): bi = lhs block slot, c0 = rhs col start,
    cw = chunk width, off = staging col offset within the pair strip."""
    gs = []
    for q in range(NPAIR):
        L = _lo_w(q)
        H = PW - L
        gs.append(dict(q=q, bi=2 * q, c0=0, cw=L, off=0))
        gs.append(dict(q=q, bi=2 * q + 1, c0=0, cw=2048, off=L))
        gs.append(dict(q=q, bi=2 * q + 1, c0=2048, cw=H - 2048,
                       off=L + 2048))
    assert len(gs) == NGRP
    return gs


def _build_nc():
    import concourse.bass as bass
    import concourse.mybir as mybir

    f32 = mybir.dt.float32
    f16 = mybir.dt.float16
    bf16 = mybir.dt.bfloat16
    Exp = mybir.ActivationFunctionType.Exp

    groups = _groups()

    nc = bass.Bass()
    xl_d = nc.dram_tensor("xl", [KA, 16 * 128], f16, kind="ExternalInput")
    xr_d = nc.dram_tensor("xr", [KA, N], f16, kind="ExternalInput")
    bias_d = nc.dram_tensor("bias", [128, 16], f32, kind="ExternalInput")
    out_d = nc.dram_tensor("out", [NPAIR * 128, PW], bf16,
                           kind="ExternalOutput")

    with (
        nc.sbuf_tensor([KA, 16 * 128], f16) as xl,
        nc.sbuf_tensor([KA, N], f16) as xr,
        nc.sbuf_tensor([128, 16], f32) as bias,
        nc.sbuf_tensor([128, NBUF_OUT * PW], bf16) as ot_buf,
        nc.psum_tensor([128, 2048], f32) as ps0,
        nc.psum_tensor([128, 2048], f32) as ps1,
        nc.semaphore("in_sem") as in_sem,
        nc.semaphore("in2_sem") as in2_sem,
        nc.semaphore("in3_sem") as in3_sem,
        nc.semaphore("mm_sem") as mm_sem,
        nc.semaphore("act_sem") as act_sem,
        nc.semaphore("out_sem_a") as out_sem_a,
        nc.semaphore("out_sem_b") as out_sem_b,
        nc.Block() as block,
    ):
        pss = [ps0, ps1]

        @block.sync
        def _(sync):
            # input loads on the SP queue in need order: xl and xr-left
            # feed group 0 (in_sem, wait 32 - completion order across
            # batches is not issue order), bias feeds the first activation
            # (in3_sem), xr-right is first needed by group 2 (in2_sem)
            sync.dma_start(out=xl[:], in_=xl_d[:, :]).then_inc(in_sem, 16)
            sync.dma_start(out=xr[:, 0:2048],
                           in_=xr_d[:, 0:2048]).then_inc(in_sem, 16)
            sync.dma_start(out=bias[:], in_=bias_d[:, :]).then_inc(in3_sem, 16)
            sync.dma_start(out=xr[:, 2048:N],
                           in_=xr_d[:, 2048:N]).then_inc(in2_sem, 16)
            for p in range(NPAIR):
                # both output rings on the otherwise-idle SP engine: cols
                # 0:L+2048 after acts g0+g1, the g2 chunk after act g2
                sync.wait_ge(act_sem, GPP * p + 2)
                L = _lo_w(p)
                s = (p % NBUF_OUT) * PW
                sync.dma_start(
                    out=out_d[p * 128 : (p + 1) * 128, 0 : L + 2048],
                    in_=ot_buf[:, s : s + L + 2048],
                ).then_inc(out_sem_a, 16)
                sync.wait_ge(act_sem, GPP * (p + 1))
                sync.dma_start(
                    out=out_d[p * 128 : (p + 1) * 128, L + 2048 : PW],
                    in_=ot_buf[:, s + L + 2048 : s + PW],
                ).then_inc(out_sem_b, 16)

        @block.tensor
        def _(tensor):
            tensor.wait_ge(in_sem, 32)
            for g, gr in enumerate(groups):
                if g == 2:
                    tensor.wait_ge(in2_sem, 16)  # xr right half loaded
                if g >= 2:
                    # psum slot g%2 was last read by group g-2's activation
                    tensor.wait_ge(act_sem, g - 1)
                ps = pss[g % 2]
                lh = xl[:, gr["bi"] * 128 : (gr["bi"] + 1) * 128]
                last = None
                for c in range(0, gr["cw"], 512):
                    last = tensor.matmul(
                        ps[:, c : c + 512],
                        lh,
                        xr[:, gr["c0"] + c : gr["c0"] + c + 512],
                        start=True, stop=True,
                    )
                last.then_inc(mm_sem, 1)

        @block.scalar
        def _(scalar):
            scalar.wait_ge(in3_sem, 16)  # bias loaded
            for g, gr in enumerate(groups):
                ps = pss[g % 2]
                scalar.wait_ge(mm_sem, g + 1)
                p = gr["q"]
                if g % GPP == 0 and p >= NBUF_OUT:
                    # staging slot p%NBUF_OUT last read by DMAs of p-NBUF_OUT
                    scalar.wait_ge(out_sem_a, 16 * (p - NBUF_OUT + 1))
                    scalar.wait_ge(out_sem_b, 16 * (p - NBUF_OUT + 1))
                s = (p % NBUF_OUT) * PW + gr["off"]
                scalar.activation(
                    ot_buf[:, s : s + gr["cw"]], ps[:, 0 : gr["cw"]], Exp,
                    bias=bias[:, gr["bi"] : gr["bi"] + 1], scale=1.0,
                ).then_inc(act_sem, 1)
    return nc


def _get_nc():
    if "nc" not in _cache:
        _cache["nc"] = _build_nc()
    return _cache["nc"]


def _prep_inputs(points: np.ndarray):
    """Host-side shard/layout prep: per-core transposed + augmented operands."""
    points = np.asarray(points, dtype=np.float32)
    per_batch = {}
    for b in range(B):
        x = points[b]                              # [N, D]
        sq = np.sum(x * x, axis=1)                 # [N]
        xt = np.ascontiguousarray(x.T)             # [D, N]
        aug_hi = (-0.5 * sq).astype(np.float16)
        aug_lo = ((-0.5 * sq) - aug_hi.astype(np.float32)).astype(np.float16)
        xr = np.empty((KA, N), np.float16)
        xr[:D] = xt
        xr[D] = aug_hi
        xr[D + 1] = aug_lo
        per_batch[b] = (xr, sq)

    in_maps = []
    for c in range(N_CORES):
        b, h = divmod(c, 2)
        xr, sq = per_batch[b]
        xl = np.empty((KA, 16 * 128), np.float16)
        bias = np.empty((128, 16), np.float32)
        for q, (r_lo, r_hi) in enumerate(_pairs(h)):
            for slot, r in ((2 * q, r_lo), (2 * q + 1, r_hi)):
                rows = slice(r * 128, (r + 1) * 128)
                xl[:D, slot * 128 : (slot + 1) * 128] = xr[:D, rows]
                bias[:, slot] = -0.5 * sq[rows]
        xl[D] = 1.0
        xl[D + 1] = 1.0
        in_maps.append({"xl": xl, "xr": xr, "bias": bias})
    return in_maps


def _assemble(results):
    """Unpack per-core strips, mirror the strict upper block triangle."""
    out = np.empty((B, N, N), np.float32)
    for c in range(N_CORES):
        b, h = divmod(c, 2)
        buf = results[c]["out"].astype(np.float32)   # [1024, PW]
        for q, (r_lo, r_hi) in enumerate(_pairs(h)):
            L = _lo_w(q)
            w_lo = (r_lo + 1) * 128
            w_hi = (r_hi + 1) * 128
            rows = buf[q * 128 : (q + 1) * 128]
            out[b, r_lo * 128 : (r_lo + 1) * 128, 0:w_lo] = rows[:, 0:w_lo]
            out[b, r_hi * 128 : (r_hi + 1) * 128, 0:w_hi] = \
                rows[:, L : L + w_hi]
    iu, ju = np.triu_indices(NBLK, 1)
    for b in range(B):
        v = out[b].reshape(NBLK, 128, NBLK, 128)
        v[iu, :, ju, :] = v[ju, :, iu, :].transpose(0, 2, 1)
    return out


def run(points: np.ndarray, **run_kwargs):
    """Run on HW; returns (K [4,4096,4096] f32, BassKernelResults)."""
    from concourse.bass_utils import run_bass_kernel_spmd

    nc = _get_nc()
    in_maps = _prep_inputs(points)
    res = run_bass_kernel_spmd(nc, in_maps, core_ids=list(range(N_CORES)),
                               **run_kwargs)
    return _assemble(res.results), res


def kernel(points: np.ndarray) -> np.ndarray:
    out, _ = run(points)
    return out


# revision 28
# speedup vs baseline: 1.2525x; 1.0529x over previous
"""Gaussian kernel matrix K = exp(-|xi-xj|^2/2) on 8 TRN2 NeuronCores,
exploiting symmetry: each core computes only lower-triangle block rows.

Input : points [4, 4096, 64] f32
Output: K      [4, 4096, 4096] f32

K[b] is symmetric: only the lower block triangle (block row r of 128
rows spans columns 0..(r+1)*128) is computed on-device; the host
mirrors the strict upper triangle.

Sharding (SPMD-uniform): core c = (batch c//2, h = c%2) takes pairs
(r, 31-r) for r = h, h+2, ..., h+14.  Widths are padded up to 512
multiples, which makes the padded pair shape IDENTICAL for all cores:
pair slot q has lo width L_q=(q//2+1)*512, hi width H_q=4608-L_q, so
every pair strip is [128, 4608] and all 8 cores run the same program
(~4.61M padded outputs/core vs 8.39M for the full-matrix split).

Math: -d2/2 = xi.xj - |xi|^2/2 - |xj|^2/2, one fp16 matmul pass with
two augmented contraction rows (ones | fp16 hi/lo of -|xj|^2/2), K=66.
-|xi|^2/2 enters as the fp32 per-partition ScalarE bias:
K = Exp(psum + bias_i), emitted bf16 (host upcasts).
"""

import numpy as np

B, N, D = 4, 4096, 64
KA = D + 2          # contraction dim: 64 dims + hi/lo aug rows
N_CORES = 8
NBLK = N // 128     # 32 block rows per batch
NPAIR = 8           # pairs per core
PW = 4608           # padded columns per pair strip (L_q + H_q)
GPP = 3             # chunk groups per pair (lo: 1, hi: 2048 + rest)
NGRP = NPAIR * GPP  # 24 pipeline groups per core

NBUF_OUT = 8        # staging strips of [128, PW] - one per pair, no reuse

_cache = {}


def _lo_w(q):
    return (q // 2 + 1) * 512   # padded lo width: 512,512,1024,...,2048


def _pairs(h):
    return [(h + 2 * q, 31 - (h + 2 * q)) for q in range(NPAIR)]


def _groups():
    """Static schedule, identical for every core: one entry per
    (pair, block, col-chunk): bi = lhs block slot, c0 = rhs col start,
    cw = chunk width, off = staging col offset within the pair strip.

    Three phases so each PE group fits inside the previous activation's
    duration (the psum ping-pong is only 2 deep): first the eight
    2048-wide hi chunks, then the lo chunks in ascending width, then the
    remaining hi chunks in descending width."""
    g1s = [dict(q=q, bi=2 * q + 1, c0=0, cw=2048, off=_lo_w(q))
           for q in range(NPAIR)]
    g0s = [dict(q=q, bi=2 * q, c0=0, cw=_lo_w(q), off=0)
           for q in range(NPAIR)]          # widths ascend with q
    g2s = [dict(q=q, bi=2 * q + 1, c0=2048, cw=PW - _lo_w(q) - 2048,
                off=_lo_w(q) + 2048)
           for q in range(NPAIR)]          # widths descend with q
    gs = g1s + g0s + g2s
    assert len(gs) == NGRP
    return gs


def _build_nc():
    import concourse.bass as bass
    import concourse.mybir as mybir

    f32 = mybir.dt.float32
    f16 = mybir.dt.float16
    bf16 = mybir.dt.bfloat16
    Exp = mybir.ActivationFunctionType.Exp

    groups = _groups()

    nc = bass.Bass()
    xl_d = nc.dram_tensor("xl", [KA, 16 * 128], f16, kind="ExternalInput")
    xr_d = nc.dram_tensor("xr", [KA, N], f16, kind="ExternalInput")
    bias_d = nc.dram_tensor("bias", [128, 16], f32, kind="ExternalInput")
    out_d = nc.dram_tensor("out", [NPAIR * 128, PW], bf16,
                           kind="ExternalOutput")

    with (
        nc.sbuf_tensor([KA, 16 * 128], f16) as xl,
        nc.sbuf_tensor([KA, N], f16) as xr,
        nc.sbuf_tensor([128, 16], f32) as bias,
        nc.sbuf_tensor([128, NBUF_OUT * PW], bf16) as ot_buf,
        nc.psum_tensor([128, 2048], f32) as ps0,
        nc.psum_tensor([128, 2048], f32) as ps1,
        nc.semaphore("in_sem") as in_sem,
        nc.semaphore("in2_sem") as in2_sem,
        nc.semaphore("in3_sem") as in3_sem,
        nc.semaphore("mm_sem") as mm_sem,
        nc.semaphore("act_sem") as act_sem,
        nc.semaphore("out_sem_a") as out_sem_a,
        nc.semaphore("out_sem_b") as out_sem_b,
        nc.Block() as block,
    ):
        pss = [ps0, ps1]

        @block.sync
        def _(sync):
            # input loads on the SP queue in need order: xl and xr-left
            # feed group 0 (in_sem, wait 32 - completion order across
            # batches is not issue order), bias feeds the first activation
            # (in3_sem), xr-right is first needed by group 2 (in2_sem)
            sync.dma_start(out=xl[:], in_=xl_d[:, :]).then_inc(in_sem, 16)
            sync.dma_start(out=xr[:, 0:2048],
                           in_=xr_d[:, 0:2048]).then_inc(in_sem, 16)
            sync.dma_start(out=bias[:], in_=bias_d[:, :]).then_inc(in3_sem, 16)
            sync.dma_start(out=xr[:, 2048:N],
                           in_=xr_d[:, 2048:N]).then_inc(in2_sem, 16)
            # both output rings on the otherwise-idle SP engine: strip
            # cols 0:L+2048 once acts g1(p)+g0(p) are done (g0(p) is act
            # number NPAIR+p+1), the g2 chunk once act 2*NPAIR+p+1 is done
            for p in range(NPAIR):
                sync.wait_ge(act_sem, NPAIR + p + 1)
                L = _lo_w(p)
                s = p * PW
                sync.dma_start(
                    out=out_d[p * 128 : (p + 1) * 128, 0 : L + 2048],
                    in_=ot_buf[:, s : s + L + 2048],
                ).then_inc(out_sem_a, 16)
            for p in range(NPAIR):
                sync.wait_ge(act_sem, 2 * NPAIR + p + 1)
                L = _lo_w(p)
                s = p * PW
                sync.dma_start(
                    out=out_d[p * 128 : (p + 1) * 128, L + 2048 : PW],
                    in_=ot_buf[:, s + L + 2048 : s + PW],
                ).then_inc(out_sem_b, 16)

        @block.tensor
        def _(tensor):
            tensor.wait_ge(in_sem, 32)
            for g, gr in enumerate(groups):
                if g == 2 * NPAIR:
                    tensor.wait_ge(in2_sem, 16)  # xr right half loaded
                if g >= 2:
                    # psum slot g%2 was last read by group g-2's activation
                    tensor.wait_ge(act_sem, g - 1)
                ps = pss[g % 2]
                lh = xl[:, gr["bi"] * 128 : (gr["bi"] + 1) * 128]
                last = None
                for c in range(0, gr["cw"], 512):
                    last = tensor.matmul(
                        ps[:, c : c + 512],
                        lh,
                        xr[:, gr["c0"] + c : gr["c0"] + c + 512],
                        start=True, stop=True,
                    )
                last.then_inc(mm_sem, 1)

        @block.scalar
        def _(scalar):
            scalar.wait_ge(in3_sem, 16)  # bias loaded
            for g, gr in enumerate(groups):
                ps = pss[g % 2]
                scalar.wait_ge(mm_sem, g + 1)
                # one staging strip per pair - no slot reuse, no waits
                s = gr["q"] * PW + gr["off"]
                scalar.activation(
                    ot_buf[:, s : s + gr["cw"]], ps[:, 0 : gr["cw"]], Exp,
                    bias=bias[:, gr["bi"] : gr["bi"] + 1], scale=1.0,
                ).then_inc(act_sem, 1)
    return nc


def _get_nc():
    if "nc" not in _cache:
        _cache["nc"] = _build_nc()
    return _cache["nc"]


def _prep_inputs(points: np.ndarray):
    """Host-side shard/layout prep: per-core transposed + augmented operands."""
    points = np.asarray(points, dtype=np.float32)
    per_batch = {}
    for b in range(B):
        x = points[b]                              # [N, D]
        sq = np.sum(x * x, axis=1)                 # [N]
        xt = np.ascontiguousarray(x.T)             # [D, N]
        aug_hi = (-0.5 * sq).astype(np.float16)
        aug_lo = ((-0.5 * sq) - aug_hi.astype(np.float32)).astype(np.float16)
        xr = np.empty((KA, N), np.float16)
        xr[:D] = xt
        xr[D] = aug_hi
        xr[D + 1] = aug_lo
        per_batch[b] = (xr, sq)

    in_maps = []
    for c in range(N_CORES):
        b, h = divmod(c, 2)
        xr, sq = per_batch[b]
        xl = np.empty((KA, 16 * 128), np.float16)
        bias = np.empty((128, 16), np.float32)
        for q, (r_lo, r_hi) in enumerate(_pairs(h)):
            for slot, r in ((2 * q, r_lo), (2 * q + 1, r_hi)):
                rows = slice(r * 128, (r + 1) * 128)
                xl[:D, slot * 128 : (slot + 1) * 128] = xr[:D, rows]
                bias[:, slot] = -0.5 * sq[rows]
        xl[D] = 1.0
        xl[D + 1] = 1.0
        in_maps.append({"xl": xl, "xr": xr, "bias": bias})
    return in_maps


def _assemble(results):
    """Unpack per-core strips, mirror the strict upper block triangle."""
    out = np.empty((B, N, N), np.float32)
    for c in range(N_CORES):
        b, h = divmod(c, 2)
        buf = results[c]["out"].astype(np.float32)   # [1024, PW]
        for q, (r_lo, r_hi) in enumerate(_pairs(h)):
            L = _lo_w(q)
            w_lo = (r_lo + 1) * 128
            w_hi = (r_hi + 1) * 128
            rows = buf[q * 128 : (q + 1) * 128]
            out[b, r_lo * 128 : (r_lo + 1) * 128, 0:w_lo] = rows[:, 0:w_lo]
            out[b, r_hi * 128 : (r_hi + 1) * 128, 0:w_hi] = \
                rows[:, L : L + w_hi]
    iu, ju = np.triu_indices(NBLK, 1)
    for b in range(B):
        v = out[b].reshape(NBLK, 128, NBLK, 128)
        v[iu, :, ju, :] = v[ju, :, iu, :].transpose(0, 2, 1)
    return out


def run(points: np.ndarray, **run_kwargs):
    """Run on HW; returns (K [4,4096,4096] f32, BassKernelResults)."""
    from concourse.bass_utils import run_bass_kernel_spmd

    nc = _get_nc()
    in_maps = _prep_inputs(points)
    res = run_bass_kernel_spmd(nc, in_maps, core_ids=list(range(N_CORES)),
                               **run_kwargs)
    return _assemble(res.results), res


def kernel(points: np.ndarray) -> np.ndarray:
    out, _ = run(points)
    return out
